# revision 19
# baseline (speedup 1.0000x reference)
"""Trainium2 Bass kernel for C = triu(triu(A) @ triu(B)), N=4096, fp32.

Math: the product of upper-triangular matrices is upper-triangular, so with
host-side triu masking of A and B the kernel needs no output masking: output
tile (m, n) (128x128 tile indices) only gets contributions from k in [m, n].

Sharding (8 cores, SPMD, one NEFF): 2D grid, 4 row-groups x 2 col-groups.
Core j = (r, c) = (j // 2, j % 2) owns row-tiles {m : m % 4 == r} (8 tiles,
1024 rows) and col-tiles {n : n % 2 == c} (16 tiles, 2048 cols). Versus 1D
row-sharding this (a) cuts per-core HBM traffic ~28MB -> ~16MB because B is
replicated to 4 cores instead of 8, and (b) cuts identical-program masking
waste because the row spread within a slot is 4 instead of 8. All cores run
the identical program; where a core's triangle is smaller than the program's
k-range, the host-side triu masking makes those matmuls accumulate zeros.

Per-core layout: owned cols are packed into 4 local supers of 512 (col-tiles
n = 8s+2i+c, i=0..3, ascending). For super s the program needs k <= 8s+7 and
row slots t with 4t <= 8s+7 (nslots = 2s+2 <= 8 = PSUM bank count).

DMA: the 16 DMA engines are one shared pool and pull per-descriptor, so
streams compete by descriptor size; all loads therefore go on a single
HW-DGE queue (sync) in global need-order, interleaving A chunks (k-major
packed triu, 8 k-tiles per chunk) with B chunks (per super: s full-width
[P,8,512] chunks + one last tile holding k=8s,8s+1 plus the 6 narrowing
tails). Everything stays SBUF-resident; each HBM byte is read once.

Compute: per super, phase 1 runs k-major over the full-width chunks (matches
the load order); phase 2 runs t-major over the last 8 ks so slots finish
staggered - each slot's psum->sbuf bf16 copy (DVE; the ACT copy path is ~9x
slower) and C store (scalar HW-DGE queue) issue immediately, pipelining the
drain instead of serializing ~8 copies after the final matmul. C is written
bf16 and upcast on the host.
"""

import sys

for _p in ("/opt/trn_rl_repo", "/root/.axon_site/_ro/trn_rl_repo"):
    if _p not in sys.path:
        sys.path.insert(0, _p)

import numpy as np

N = 4096
P = 128
KT = N // P  # 32 k-tiles
NCORES = 8
RG = 4  # row groups
CG = 2  # col groups
NSLOT = KT // RG  # 8 row-tiles per core
NSUP = 4  # local col supers per core
SW = 512  # super width (cols)

# widths of the tail matmuls (k = 8s+2+j): union over c of cols >= k
TAILW = [384, 384, 256, 256, 128, 128]
TAILOFF = [0, 384, 768, 1024, 1280, 1408]
TAILSZ = 1536

# nslots at k: slots t with 4t <= k (capped at 8)
_NSK = [min(k // 4 + 1, NSLOT) for k in range(KT)]
# k-major packed A offsets (in 128-wide units)
OFFK = [0] * (KT + 1)
for _k in range(KT):
    OFFK[_k + 1] = OFFK[_k] + _NSK[_k]
ATOT = OFFK[KT]  # 144

# A load chunk boundaries (k-tiles). Each DMA pays ~2.3us of fixed cost
# (config + DGE delay + sem propagation), so chunks are as coarse as the
# need-times allow.
ACHB = [0, 8, 16, 24, 32]

# B per super: s full-width chunks of [P, 8, SW] (k in [0, 8s)), then one
# "last" tile [P, 2*SW + TAILSZ] holding k = 8s, 8s+1 full width + 6 tails
LASTSZ = 2 * SW + TAILSZ  # 2560
SZSUP = [8 * s * SW + LASTSZ for s in range(NSUP)]
BOFF = [0] * (NSUP + 1)
for _s in range(NSUP):
    BOFF[_s + 1] = BOFF[_s] + SZSUP[_s]
BTOT = BOFF[NSUP]  # 34816


def _width(s, k):
    """matmul free width at (super s, k): cols the program still covers."""
    if k < 8 * s + 2:
        return SW
    return TAILW[k - (8 * s + 2)]


_cache = {}


def _build():
    import concourse.bacc as bacc
    import concourse.mybir as mybir
    import concourse.tile as tile

    D = mybir.dt.bfloat16
    f32 = mybir.dt.float32

    nc = bacc.Bacc(None, target_bir_lowering=False)
    # A packed k-major, lhsT layout: AT[p, OFFK[k]+t, ml] = Au[(4t+r)*128+ml, k*128+p]
    Am = nc.dram_tensor("AT", [P, ATOT, P], D, kind="ExternalInput")
    # B packed per super (full chunks + last tile), per-partition contiguous
    Bm = nc.dram_tensor("B", [P, BTOT], D, kind="ExternalInput")
    # C rows: slot-major (8*128), cols: super-major (4*512), bf16
    Cm = nc.dram_tensor("C", [NSLOT * P, NSUP * SW], D, kind="ExternalOutput")

    with tile.TileContext(nc) as tc:
        with (
            tc.tile_pool(name="a", bufs=1) as apool,
            tc.tile_pool(name="b", bufs=1) as bpool,
            tc.tile_pool(name="o", bufs=8) as opool,
            tc.tile_pool(name="ps", bufs=8, space="PSUM") as pspool,
        ):
            a_tiles = [None] * (len(ACHB) - 1)
            bf8_tiles = [[None] * s for s in range(NSUP)]
            blast_tiles = [None] * NSUP  # s=0 handled by b0k01/b0t below

            # Loads split across the two HW-DGE queues (SP + ACT): the DMA
            # engines pull from both queues concurrently, so two FIFOs give
            # ~1.4x one queue's bandwidth. Each queue's own order follows the
            # compute's need-order; sizes are balanced (~7MB each) so the
            # queue heads advance together. Stores later round-robin too.
            _qs = [nc.sync, nc.scalar]
            _qi = [0]

            def _q():
                e = _qs[_qi[0] % 2]
                _qi[0] += 1
                return e

            def load_a(g, eng):
                w = OFFK[ACHB[g + 1]] - OFFK[ACHB[g]]
                ag = apool.tile([P, w, P], D, tag=f"a{g}", name="ag")
                eng.dma_start(ag[:], Am[:, OFFK[ACHB[g]] : OFFK[ACHB[g + 1]], :])
                a_tiles[g] = ag

            def load_bf8(s, i, eng):
                bt = bpool.tile([P, 8, SW], D, tag=f"b{s}f{i}", name="bt")
                o = BOFF[s] + 8 * i * SW
                eng.dma_start(bt[:], Bm[:, o : o + 8 * SW])
                bf8_tiles[s][i] = bt

            def load_blast(s, eng):
                bt = bpool.tile([P, LASTSZ], D, tag=f"b{s}l", name="bl")
                o = BOFF[s] + 8 * s * SW
                eng.dma_start(bt[:], Bm[:, o : o + LASTSZ])
                blast_tiles[s] = bt

            # interleave emission so each queue's FIFO is its need-order
            load_blast(0, nc.sync)  # s0's whole B (k0,k1 + tails)
            load_a(0, nc.scalar)  # k0-7
            load_bf8(1, 0, nc.sync)
            load_a(1, nc.scalar)  # k8-15
            load_blast(1, nc.sync)
            load_bf8(2, 0, nc.scalar)
            load_bf8(2, 1, nc.sync)
            load_a(2, nc.scalar)  # k16-23
            load_blast(2, nc.sync)
            load_bf8(3, 0, nc.scalar)
            load_bf8(3, 1, nc.sync)
            load_bf8(3, 2, nc.scalar)
            load_a(3, nc.sync)  # k24-31
            load_blast(3, nc.scalar)

            # warm-up: the first real matmul can't start until the first
            # loads land (~5us); spend that window running throwaway matmuls
            # so the PE p-state ramp (0.65 -> 1.2 -> 2.4 GHz over ~3us of
            # continuous busy) completes before real work arrives. They
            # target super-0's psum bank as standalone start/stop groups;
            # the real chain re-starts the bank so WAW order is enough.
            warm = apool.tile([P, SW], D, tag="warm", name="warm")
            nc.gpsimd.memset(warm[:], 0)
            ps0 = [
                pspool.tile([P, SW], f32, tag="ps", name="ps") for _ in range(2)
            ]
            # sized so the PE stays busy until the first loads land (~12us):
            # ~7 ramping matmuls (~3us) then warm ones
            for _ in range(12):
                nc.tensor.matmul(
                    ps0[0][:], warm[:, :P], warm[:], start=True, stop=True
                )

            from bisect import bisect_right

            def lhs(k, t):
                g = bisect_right(ACHB, k) - 1
                return a_tiles[g][:, OFFK[k] - OFFK[ACHB[g]] + t, :]

            # known DMA-wait at each super boundary (early supers consume
            # bytes faster than the stream supplies them); bridge it with
            # filler matmuls so the PE p-state ramp never resets. They
            # target the next super's LAST slot's psum, whose first real
            # matmul is mid-super, so the WAW ordering costs nothing.
            NDUM = {1: 6, 2: 2}

            for s in range(NSUP):
                kmax = 8 * s + 7
                ns = 2 * s + 2
                psums = ps0 if s == 0 else [
                    pspool.tile([P, SW], f32, tag="ps", name="ps")
                    for _ in range(ns)
                ]
                for _ in range(NDUM.get(s, 0)):
                    nc.tensor.matmul(
                        psums[ns - 1][:], warm[:, :P], warm[:],
                        start=True, stop=True,
                    )

                # phase 1: k-major over the full-width chunks (load order)
                for k in range(8 * s):
                    rhs = bf8_tiles[s][k // 8][:, k % 8, :]
                    for t in range(k // 4 + 1):
                        nc.tensor.matmul(
                            psums[t][:],
                            lhs(k, t),
                            rhs,
                            start=(k == 4 * t),
                            stop=False,
                        )
                # phase 2: t-major over the last 8 ks; slots finish staggered
                # so the copy+store drain pipelines with remaining matmuls
                bl = blast_tiles[s]
                for t in range(ns):
                    for k in range(max(4 * t, 8 * s), 8 * s + 8):
                        w = _width(s, k)
                        j = k - 8 * s
                        if j < 2:
                            rhs = bl[:, j * SW : (j + 1) * SW]
                        else:
                            o = 2 * SW + TAILOFF[j - 2]
                            rhs = bl[:, o : o + w]
                        nc.tensor.matmul(
                            psums[t][:, SW - w : SW],
                            lhs(k, t),
                            rhs,
                            start=(k == 4 * t),
                            stop=(k == kmax),
                        )
                    w0 = SW - _width(s, 4 * t)
                    ot = opool.tile([P, SW], D, tag="o", name="ot")
                    nc.vector.tensor_copy(ot[:, w0:SW], psums[t][:, w0:SW])
                    # stores continue the queue round-robin: both HW queues
                    # are free of loads by drain time, halving the tail
                    _q().dma_start(
                        Cm[P * t : P * (t + 1), SW * s + w0 : SW * (s + 1)],
                        ot[:, w0:SW],
                    )
    nc.compile()
    return nc


def _get_nc():
    if "nc" not in _cache:
        _cache["nc"] = _build()
    return _cache["nc"]


def _np_bf16():
    import ml_dtypes

    return np.dtype(ml_dtypes.bfloat16)


def _make_in_maps(A, B):
    A = np.asarray(A, dtype=np.float32)
    B = np.asarray(B, dtype=np.float32)
    Au = np.triu(A)
    Bu = np.triu(B)
    bf16 = _np_bf16()

    Au4 = Au.reshape(KT, P, KT, P)  # [mt, ml, kt, p]
    Bu4 = Bu.reshape(KT, P, KT, P)  # [kt, p, nt, q]

    # A payload depends only on r; B payload only on c
    A_r = []
    for r in range(RG):
        ATd = np.empty((P, ATOT, P), dtype=bf16)
        for k in range(KT):
            for t in range(_NSK[k]):
                # lhsT tile: [p, ml] = Au[(4t+r)*128+ml, k*128+p]
                ATd[:, OFFK[k] + t, :] = Au4[4 * t + r, :, k, :].T
        A_r.append(ATd)

    B_c = []
    for c in range(CG):
        segs = []
        for s in range(NSUP):
            nt0 = 8 * s + c
            # full-width region: k < 8s+2, all 4 owned col-tiles of the super
            full = Bu4[: 8 * s + 2, :, nt0 : nt0 + 8 : 2, :]  # [K, p, 4, q]
            segs.append(
                np.ascontiguousarray(full.transpose(1, 0, 2, 3)).reshape(
                    P, (8 * s + 2) * SW
                )
            )
            for j, w in enumerate(TAILW):
                k = 8 * s + 2 + j
                i0 = 4 - w // P
                tail = Bu4[k, :, nt0 + 2 * i0 : nt0 + 8 : 2, :]  # [p, 4-i0, q]
                segs.append(np.ascontiguousarray(tail).reshape(P, w))
        B_c.append(np.concatenate(segs, axis=1).astype(bf16))

    in_maps = []
    for j in range(NCORES):
        r, c = j // CG, j % CG
        in_maps.append({"AT": A_r[r], "B": B_c[c]})
    return in_maps


def kernel(A, B):
    from concourse.bass_utils import run_bass_kernel_spmd

    in_maps = _make_in_maps(A, B)
    nc = _get_nc()
    res = run_bass_kernel_spmd(nc, in_maps, core_ids=list(range(NCORES)))

    C4 = np.zeros((KT, P, KT, P), dtype=np.float32)
    for j in range(NCORES):
        r, c = j // CG, j % CG
        Cj = np.asarray(res.results[j]["C"]).astype(np.float32)
        # rows: slot-major (t -> row-tile 4t+r); cols: (s, i) -> col-tile
        # 8s+2i+c, which is exactly c::2 in ascending order
        C4[r::RG, :, c::CG, :] = Cj.reshape(NSLOT, P, KT // CG, P)
    C = C4.reshape(N, N)
    # below-diagonal tiles the program never stores are uninitialized; the
    # triu kills them (diag-tile interiors are exact zeros from the masking)
    return np.triu(C)


# revision 20
# speedup vs baseline: 1.0095x; 1.0095x over previous
"""Trainium2 Bass kernel for C = triu(triu(A) @ triu(B)), N=4096, fp32.

Math: the product of upper-triangular matrices is upper-triangular, so with
host-side triu masking of A and B the kernel needs no output masking: output
tile (m, n) (128x128 tile indices) only gets contributions from k in [m, n].

Sharding (8 cores, SPMD, one NEFF): 2D grid, 4 row-groups x 2 col-groups.
Core j = (r, c) = (j // 2, j % 2) owns row-tiles {m : m % 4 == r} (8 tiles,
1024 rows) and col-tiles {n : n % 2 == c} (16 tiles, 2048 cols). Versus 1D
row-sharding this (a) cuts per-core HBM traffic ~28MB -> ~16MB because B is
replicated to 4 cores instead of 8, and (b) cuts identical-program masking
waste because the row spread within a slot is 4 instead of 8. All cores run
the identical program; where a core's triangle is smaller than the program's
k-range, the host-side triu masking makes those matmuls accumulate zeros.

Per-core layout: owned cols are packed into 4 local supers of 512 (col-tiles
n = 8s+2i+c, i=0..3, ascending). For super s the program needs k <= 8s+7 and
row slots t with 4t <= 8s+7 (nslots = 2s+2 <= 8 = PSUM bank count).

DMA: the 16 DMA engines are one shared pool and pull per-descriptor, so
streams compete by descriptor size; all loads therefore go on a single
HW-DGE queue (sync) in global need-order, interleaving A chunks (k-major
packed triu, 8 k-tiles per chunk) with B chunks (per super: s full-width
[P,8,512] chunks + one last tile holding k=8s,8s+1 plus the 6 narrowing
tails). Everything stays SBUF-resident; each HBM byte is read once.

Compute: per super, phase 1 runs k-major over the full-width chunks (matches
the load order); phase 2 runs t-major over the last 8 ks so slots finish
staggered - each slot's psum->sbuf bf16 copy (DVE; the ACT copy path is ~9x
slower) and C store (scalar HW-DGE queue) issue immediately, pipelining the
drain instead of serializing ~8 copies after the final matmul. C is written
bf16 and upcast on the host.
"""

import sys

for _p in ("/opt/trn_rl_repo", "/root/.axon_site/_ro/trn_rl_repo"):
    if _p not in sys.path:
        sys.path.insert(0, _p)

import numpy as np

N = 4096
P = 128
KT = N // P  # 32 k-tiles
NCORES = 8
RG = 4  # row groups
CG = 2  # col groups
NSLOT = KT // RG  # 8 row-tiles per core
NSUP = 4  # local col supers per core
SW = 512  # super width (cols)

# widths of the tail matmuls (k = 8s+2+j): union over c of cols >= k
TAILW = [384, 384, 256, 256, 128, 128]
TAILOFF = [0, 384, 768, 1024, 1280, 1408]
TAILSZ = 1536

# nslots at k: slots t with 4t <= k (capped at 8)
_NSK = [min(k // 4 + 1, NSLOT) for k in range(KT)]
# k-major packed A offsets (in 128-wide units)
OFFK = [0] * (KT + 1)
for _k in range(KT):
    OFFK[_k + 1] = OFFK[_k] + _NSK[_k]
ATOT = OFFK[KT]  # 144

# A load chunk boundaries (k-tiles). Each DMA pays ~2.3us of fixed cost
# (config + DGE delay + sem propagation), so chunks are as coarse as the
# need-times allow.
ACHB = [0, 8, 16, 24, 32]

# B per super: s full-width chunks of [P, 8, SW] (k in [0, 8s)), then one
# "last" tile [P, 2*SW + TAILSZ] holding k = 8s, 8s+1 full width + 6 tails
LASTSZ = 2 * SW + TAILSZ  # 2560
SZSUP = [8 * s * SW + LASTSZ for s in range(NSUP)]
BOFF = [0] * (NSUP + 1)
for _s in range(NSUP):
    BOFF[_s + 1] = BOFF[_s] + SZSUP[_s]
BTOT = BOFF[NSUP]  # 34816


def _width(s, k):
    """matmul free width at (super s, k): cols the program still covers."""
    if k < 8 * s + 2:
        return SW
    return TAILW[k - (8 * s + 2)]


_cache = {}


def _build():
    import concourse.bacc as bacc
    import concourse.mybir as mybir
    import concourse.tile as tile

    D = mybir.dt.bfloat16
    f32 = mybir.dt.float32

    nc = bacc.Bacc(None, target_bir_lowering=False)
    # A packed k-major, lhsT layout: AT[p, OFFK[k]+t, ml] = Au[(4t+r)*128+ml, k*128+p]
    Am = nc.dram_tensor("AT", [P, ATOT, P], D, kind="ExternalInput")
    # B packed per super (full chunks + last tile), per-partition contiguous
    Bm = nc.dram_tensor("B", [P, BTOT], D, kind="ExternalInput")
    # C rows: slot-major (8*128), cols: super-major (4*512), bf16
    Cm = nc.dram_tensor("C", [NSLOT * P, NSUP * SW], D, kind="ExternalOutput")

    with tile.TileContext(nc) as tc:
        with (
            tc.tile_pool(name="a", bufs=1) as apool,
            tc.tile_pool(name="b", bufs=1) as bpool,
            tc.tile_pool(name="o", bufs=8) as opool,
            tc.tile_pool(name="ps", bufs=8, space="PSUM") as pspool,
        ):
            a_tiles = [None] * (len(ACHB) - 1)
            bf8_tiles = [[None] * s for s in range(NSUP)]
            blast_tiles = [None] * NSUP  # s=0 handled by b0k01/b0t below

            # Loads split across the two HW-DGE queues (SP + ACT): the DMA
            # engines pull from both queues concurrently, so two FIFOs give
            # ~1.4x one queue's bandwidth. Each queue's own order follows the
            # compute's need-order; sizes are balanced (~7MB each) so the
            # queue heads advance together. Stores later round-robin too.
            _qs = [nc.sync, nc.scalar]
            _qi = [0]

            def _q():
                e = _qs[_qi[0] % 2]
                _qi[0] += 1
                return e

            def load_a(g, eng):
                w = OFFK[ACHB[g + 1]] - OFFK[ACHB[g]]
                ag = apool.tile([P, w, P], D, tag=f"a{g}", name="ag")
                eng.dma_start(ag[:], Am[:, OFFK[ACHB[g]] : OFFK[ACHB[g + 1]], :])
                a_tiles[g] = ag

            def load_bf8(s, i, eng):
                bt = bpool.tile([P, 8, SW], D, tag=f"b{s}f{i}", name="bt")
                o = BOFF[s] + 8 * i * SW
                eng.dma_start(bt[:], Bm[:, o : o + 8 * SW])
                bf8_tiles[s][i] = bt

            def load_blast(s, eng):
                bt = bpool.tile([P, LASTSZ], D, tag=f"b{s}l", name="bl")
                o = BOFF[s] + 8 * s * SW
                eng.dma_start(bt[:], Bm[:, o : o + LASTSZ])
                blast_tiles[s] = bt

            # interleave emission so each queue's FIFO is its need-order
            load_blast(0, nc.sync)  # s0's whole B (k0,k1 + tails)
            load_a(0, nc.scalar)  # k0-7
            load_bf8(1, 0, nc.sync)
            load_a(1, nc.scalar)  # k8-15
            load_blast(1, nc.sync)
            load_bf8(2, 0, nc.scalar)
            load_bf8(2, 1, nc.sync)
            load_a(2, nc.scalar)  # k16-23
            load_blast(2, nc.sync)
            load_bf8(3, 0, nc.scalar)
            load_bf8(3, 1, nc.sync)
            load_bf8(3, 2, nc.scalar)
            load_a(3, nc.sync)  # k24-31
            load_blast(3, nc.scalar)

            # warm-up: the first real matmul can't start until the first
            # loads land (~5us); spend that window running throwaway matmuls
            # so the PE p-state ramp (0.65 -> 1.2 -> 2.4 GHz over ~3us of
            # continuous busy) completes before real work arrives. They
            # target super-0's psum bank as standalone start/stop groups;
            # the real chain re-starts the bank so WAW order is enough.
            warm = apool.tile([P, SW], D, tag="warm", name="warm")
            nc.gpsimd.memset(warm[:], 0)
            ps0 = [
                pspool.tile([P, SW], f32, tag="ps", name="ps") for _ in range(2)
            ]
            # sized so the PE stays busy until the first loads land (~12us):
            # ~7 ramping matmuls (~3us) then warm ones
            for _ in range(12):
                nc.tensor.matmul(
                    ps0[0][:], warm[:, :P], warm[:], start=True, stop=True
                )

            from bisect import bisect_right

            def lhs(k, t):
                g = bisect_right(ACHB, k) - 1
                return a_tiles[g][:, OFFK[k] - OFFK[ACHB[g]] + t, :]

            # known DMA-wait at each super boundary (early supers consume
            # bytes faster than the stream supplies them); bridge it with
            # filler matmuls so the PE p-state ramp never resets. They
            # target the next super's LAST slot's psum, whose first real
            # matmul is mid-super, so the WAW ordering costs nothing.
            NDUM = {1: 9, 2: 5}

            for s in range(NSUP):
                kmax = 8 * s + 7
                ns = 2 * s + 2
                psums = ps0 if s == 0 else [
                    pspool.tile([P, SW], f32, tag="ps", name="ps")
                    for _ in range(ns)
                ]
                for _ in range(NDUM.get(s, 0)):
                    nc.tensor.matmul(
                        psums[ns - 1][:], warm[:, :P], warm[:],
                        start=True, stop=True,
                    )

                # phase 1: k-major over the full-width chunks (load order)
                for k in range(8 * s):
                    rhs = bf8_tiles[s][k // 8][:, k % 8, :]
                    for t in range(k // 4 + 1):
                        nc.tensor.matmul(
                            psums[t][:],
                            lhs(k, t),
                            rhs,
                            start=(k == 4 * t),
                            stop=False,
                        )
                # phase 2: t-major over the last 8 ks; slots finish staggered
                # so the copy+store drain pipelines with remaining matmuls
                bl = blast_tiles[s]
                for t in range(ns):
                    for k in range(max(4 * t, 8 * s), 8 * s + 8):
                        w = _width(s, k)
                        j = k - 8 * s
                        if j < 2:
                            rhs = bl[:, j * SW : (j + 1) * SW]
                        else:
                            o = 2 * SW + TAILOFF[j - 2]
                            rhs = bl[:, o : o + w]
                        nc.tensor.matmul(
                            psums[t][:, SW - w : SW],
                            lhs(k, t),
                            rhs,
                            start=(k == 4 * t),
                            stop=(k == kmax),
                        )
                    w0 = SW - _width(s, 4 * t)
                    ot = opool.tile([P, SW], D, tag="o", name="ot")
                    nc.vector.tensor_copy(ot[:, w0:SW], psums[t][:, w0:SW])
                    # stores continue the queue round-robin: both HW queues
                    # are free of loads by drain time, halving the tail
                    _q().dma_start(
                        Cm[P * t : P * (t + 1), SW * s + w0 : SW * (s + 1)],
                        ot[:, w0:SW],
                    )
    nc.compile()
    return nc


def _get_nc():
    if "nc" not in _cache:
        _cache["nc"] = _build()
    return _cache["nc"]


def _np_bf16():
    import ml_dtypes

    return np.dtype(ml_dtypes.bfloat16)


def _make_in_maps(A, B):
    A = np.asarray(A, dtype=np.float32)
    B = np.asarray(B, dtype=np.float32)
    Au = np.triu(A)
    Bu = np.triu(B)
    bf16 = _np_bf16()

    Au4 = Au.reshape(KT, P, KT, P)  # [mt, ml, kt, p]
    Bu4 = Bu.reshape(KT, P, KT, P)  # [kt, p, nt, q]

    # A payload depends only on r; B payload only on c
    A_r = []
    for r in range(RG):
        ATd = np.empty((P, ATOT, P), dtype=bf16)
        for k in range(KT):
            for t in range(_NSK[k]):
                # lhsT tile: [p, ml] = Au[(4t+r)*128+ml, k*128+p]
                ATd[:, OFFK[k] + t, :] = Au4[4 * t + r, :, k, :].T
        A_r.append(ATd)

    B_c = []
    for c in range(CG):
        segs = []
        for s in range(NSUP):
            nt0 = 8 * s + c
            # full-width region: k < 8s+2, all 4 owned col-tiles of the super
            full = Bu4[: 8 * s + 2, :, nt0 : nt0 + 8 : 2, :]  # [K, p, 4, q]
            segs.append(
                np.ascontiguousarray(full.transpose(1, 0, 2, 3)).reshape(
                    P, (8 * s + 2) * SW
                )
            )
            for j, w in enumerate(TAILW):
                k = 8 * s + 2 + j
                i0 = 4 - w // P
                tail = Bu4[k, :, nt0 + 2 * i0 : nt0 + 8 : 2, :]  # [p, 4-i0, q]
                segs.append(np.ascontiguousarray(tail).reshape(P, w))
        B_c.append(np.concatenate(segs, axis=1).astype(bf16))

    in_maps = []
    for j in range(NCORES):
        r, c = j // CG, j % CG
        in_maps.append({"AT": A_r[r], "B": B_c[c]})
    return in_maps


def kernel(A, B):
    from concourse.bass_utils import run_bass_kernel_spmd

    in_maps = _make_in_maps(A, B)
    nc = _get_nc()
    res = run_bass_kernel_spmd(nc, in_maps, core_ids=list(range(NCORES)))

    C4 = np.zeros((KT, P, KT, P), dtype=np.float32)
    for j in range(NCORES):
        r, c = j // CG, j % CG
        Cj = np.asarray(res.results[j]["C"]).astype(np.float32)
        # rows: slot-major (t -> row-tile 4t+r); cols: (s, i) -> col-tile
        # 8s+2i+c, which is exactly c::2 in ascending order
        C4[r::RG, :, c::CG, :] = Cj.reshape(NSLOT, P, KT // CG, P)
    C = C4.reshape(N, N)
    # below-diagonal tiles the program never stores are uninitialized; the
    # triu kills them (diag-tile interiors are exact zeros from the masking)
    return np.triu(C)
